# revision 1
# baseline (speedup 1.0000x reference)
"""Trainium2 Bass kernel for: 1x1-conv GEMM + GroupNorm + HardTanh.

Reference computation (per sample b):
    y = weight @ x[b]                        # [512, 256] @ [256, 56*56]
    groupnorm over 32 groups of 16 channels  # stats over (16, 56*56)
    y = y * gamma + beta                     # per-channel affine
    out = clip(y, -2, 2)

Sharding: data-parallel over batch, 4 samples per core x 8 cores.
weight/gamma/beta replicated. No cross-core communication needed.

Matmul runs in float32r (full PE rate for N>=256, ~1e-3 accuracy vs
4x slower plain fp32; measured output error 4.3e-4 of scale).
Per-partition GroupNorm stats come from bn_stats/bn_aggr reading PSUM
directly; the 16-partition group reduction AND broadcast back to all
partitions is ONE tiny PE matmul against a block-diagonal 1/16 matrix,
written into the spare tail columns of the last matmul tile's PSUM
bank (so all 8 banks stay available to matmul tiles and the next
chunk's matmuls overlap the current chunk's normalization chain).
Engine assignment per 128-channel chunk: PE matmuls -> DVE bn_stats
(from PSUM) -> tiny DVE/ACT chain for rstd/scale/bias -> ACT affine
per tile (frees PSUM banks incrementally) -> Pool clamp -> store.
x loads are split into column-range quarters and prefetched two
samples deep so the SP DMA FIFO never starves the matmuls.
"""

import sys

sys.path.insert(0, "/opt/trn_rl_repo")

import numpy as np

import concourse.bacc as bacc
import concourse.mybir as mybir
import concourse.tile as tile
from concourse.bass_utils import run_bass_kernel_spmd

# Problem shape (hardcoded per contest contract)
B, CIN, COUT, H, W = 32, 256, 512, 56, 56
HW = H * W  # 3136
G = 32  # num groups
GSIZE = COUT // G  # 16 channels per group
EPS = 1e-5
HT_MIN, HT_MAX = -2.0, 2.0

N_CORES = 8
BPC = B // N_CORES  # samples per core = 4
KC = CIN // 128  # contraction chunks = 2
OC = COUT // 128  # output-channel chunks = 4
NT = 7  # free-dim tiles per row
NTS = HW // NT  # 448 per tile (one PSUM bank, fp32)

_NC_CACHE = None


def _build_program():
    f32 = mybir.dt.float32
    f32r = mybir.dt.float32r

    nc = bacc.Bacc("TRN2", target_bir_lowering=False, debug=False)

    x_d = nc.dram_tensor("x", [BPC, CIN, HW], f32r, kind="ExternalInput")
    wt_d = nc.dram_tensor("wt", [CIN, COUT], f32r, kind="ExternalInput")
    gamma_d = nc.dram_tensor("gamma", [COUT], f32, kind="ExternalInput")
    beta_d = nc.dram_tensor("beta", [COUT], f32, kind="ExternalInput")
    agg_d = nc.dram_tensor("agg", [128, 128], f32, kind="ExternalInput")
    out_d = nc.dram_tensor("out", [BPC, COUT, HW], f32, kind="ExternalOutput")

    with tile.TileContext(nc) as tc:
        with (
            tc.tile_pool(name="singles", bufs=1) as singles,
            tc.tile_pool(name="xp", bufs=4) as xp,
            tc.tile_pool(name="op", bufs=3) as op,
            tc.tile_pool(name="small", bufs=4) as small,
            tc.tile_pool(name="psy", bufs=8, space="PSUM") as psy,
        ):
            # --- one-time setup -------------------------------------------
            # first sample's first x quarter goes FIRST on HWDGE so the
            # first matmul can start ASAP; scalars ride SWDGE (gpsimd)
            XQ = 4  # x loaded in 4 column-range DMAs so matmuls start early
            QW = HW // XQ  # 784
            x0_sb = xp.tile([128, KC, HW], f32r, tag="x")
            nc.sync.dma_start(
                out=x0_sb[:, :, 0:QW],
                in_=x_d.ap()[0, :, 0:QW].rearrange("(c p) f -> p c f", p=128),
            )
            wt_sb = singles.tile([128, KC, COUT], f32r)
            nc.sync.dma_start(
                out=wt_sb, in_=wt_d.ap().rearrange("(c p) m -> p c m", p=128)
            )
            gamma_sb = singles.tile([128, OC], f32)
            nc.gpsimd.dma_start(
                out=gamma_sb, in_=gamma_d.ap().rearrange("(c p) -> p c", p=128)
            )
            beta_sb = singles.tile([128, OC], f32)
            nc.gpsimd.dma_start(
                out=beta_sb, in_=beta_d.ap().rearrange("(c p) -> p c", p=128)
            )
            eps_sb = singles.tile([128, 1], f32)
            nc.vector.memset(eps_sb, EPS)
            agg_sb = singles.tile([128, 128], f32)
            nc.gpsimd.dma_start(out=agg_sb, in_=agg_d.ap())

            # --- main loop ------------------------------------------------
            def load_x_quarter(x_tile, b, q):
                qsl = slice(q * QW, (q + 1) * QW)
                nc.sync.dma_start(
                    out=x_tile[:, :, qsl],
                    in_=x_d.ap()[b, :, qsl].rearrange("(c p) f -> p c f", p=128),
                )

            x_tiles = [x0_sb]
            for q in range(1, XQ):
                load_x_quarter(x0_sb, 0, q)

            for b in range(BPC):
                x_sb = x_tiles[b]
                for oc in range(OC):
                    # spread next sample's x-load quarters between chunks so
                    # they enter the SP DMA FIFO ahead of later stores
                    if b + 1 < BPC and oc < 2:
                        if oc == 0:
                            xnext = xp.tile([128, KC, HW], f32r, tag="x")
                            x_tiles.append(xnext)
                        for j in range(XQ // 2):
                            load_x_quarter(
                                x_tiles[b + 1], b + 1, (XQ // 2) * oc + j
                            )
                    osl = slice(oc * 128, (oc + 1) * 128)
                    st = small.tile([128, NT, 6], f32, tag="st")

                    ps_tiles = []
                    for nt in range(NT):
                        nsl = slice(nt * NTS, (nt + 1) * NTS)
                        ps = psy.tile([128, 512], f32, tag="ymm")
                        ps_tiles.append(ps)
                        for c in range(KC):
                            nc.tensor.matmul(
                                ps[:, 0:NTS],
                                wt_sb[:, c, osl],
                                x_sb[:, c, nsl],
                                start=(c == 0),
                                stop=(c == KC - 1),
                            )
                        nc.vector.bn_stats(out=st[:, nt, :], in_=ps[:, 0:NTS])

                    # per-partition stats: stat3 = [mean, var, mean^2]
                    stat3 = small.tile([128, 3], f32, tag="stat3")
                    nc.vector.bn_aggr(out=stat3[:, 0:2], in_=st)
                    nc.vector.tensor_mul(stat3[:, 2:3], stat3[:, 0:1], stat3[:, 0:1])

                    # group-aggregate + broadcast in one matmul, written into
                    # the unused tail columns of the LAST tile's PSUM bank
                    # (that bank lives longest anyway):
                    # gps[p, j] = avg over p' in group(p) of stat3[p', j]
                    gps = ps_tiles[NT - 1][:, NTS : NTS + 3]
                    nc.tensor.matmul(
                        gps, agg_sb, stat3, start=True, stop=True,
                        skip_group_check=True,
                    )
                    gs = small.tile([128, 3], f32, tag="gs")
                    nc.vector.tensor_copy(out=gs, in_=gps)

                    # group var = E[var] + E[m^2] - mean_g^2
                    # sd = sqrt(var_g + eps); rstd = 1/sd
                    msq = small.tile([128, 1], f32, tag="msq")
                    nc.vector.tensor_mul(msq, gs[:, 0:1], gs[:, 0:1])
                    sd = small.tile([128, 1], f32, tag="sd")
                    nc.vector.tensor_scalar(
                        out=sd,
                        in0=gs[:, 1:2],
                        scalar1=gs[:, 2:3],
                        scalar2=msq,
                        op0=mybir.AluOpType.add,
                        op1=mybir.AluOpType.subtract,
                    )
                    nc.scalar.activation(
                        out=sd,
                        in_=sd,
                        func=mybir.ActivationFunctionType.Sqrt,
                        bias=eps_sb,
                    )
                    rstd = small.tile([128, 1], f32, tag="rstd")
                    nc.vector.reciprocal(rstd, sd)

                    # s = rstd*gamma ; bv = beta - mean*s
                    s = small.tile([128, 1], f32, tag="s")
                    nc.vector.tensor_mul(s, rstd, gamma_sb[:, oc : oc + 1])
                    ms = small.tile([128, 1], f32, tag="ms")
                    nc.vector.tensor_mul(ms, gs[:, 0:1], s)
                    bv = small.tile([128, 1], f32, tag="bv")
                    nc.vector.tensor_sub(bv, beta_sb[:, oc : oc + 1], ms)

                    # yn = y*s + bv per tile (ACT, reads PSUM, frees banks
                    # incrementally); clamp on Pool; store pairs
                    yn_sb = op.tile([128, HW], f32, tag="yn")
                    for nt in range(NT):
                        nsl = slice(nt * NTS, (nt + 1) * NTS)
                        nc.scalar.activation(
                            out=yn_sb[:, nsl],
                            in_=ps_tiles[nt][:, 0:NTS],
                            func=mybir.ActivationFunctionType.Identity,
                            bias=bv,
                            scale=s,
                        )
                        last_chunk = b == BPC - 1 and oc == OC - 1
                        if last_chunk:
                            # drain the final chunk per tile so the last
                            # store finishes right after the last affine
                            flush = [(nt, nt * NTS)]
                        elif nt in (1, 3, 5, NT - 1):
                            flush = [(nt, {1: 0, 3: 2, 5: 4, NT - 1: 6}[nt] * NTS)]
                        else:
                            flush = []
                        for _nt, lo in flush:
                            hsl = slice(lo, (_nt + 1) * NTS)
                            nc.gpsimd.tensor_scalar(
                                out=yn_sb[:, hsl],
                                in0=yn_sb[:, hsl],
                                scalar1=HT_MAX,
                                scalar2=HT_MIN,
                                op0=mybir.AluOpType.min,
                                op1=mybir.AluOpType.max,
                            )
                            nc.sync.dma_start(
                                out=out_d.ap()[b, osl, hsl], in_=yn_sb[:, hsl]
                            )

    nc.compile()
    return nc


def _get_program():
    global _NC_CACHE
    if _NC_CACHE is None:
        _NC_CACHE = _build_program()
    return _NC_CACHE


def _make_in_maps(x, weight, gamma, beta):
    xr = np.ascontiguousarray(x.reshape(B, CIN, HW))
    wt = np.ascontiguousarray(weight.T)  # [CIN, COUT]
    gamma = np.ascontiguousarray(gamma, dtype=np.float32)
    beta = np.ascontiguousarray(beta, dtype=np.float32)
    agg = np.zeros((128, 128), dtype=np.float32)
    for g in range(128 // GSIZE):
        agg[g * GSIZE : (g + 1) * GSIZE, g * GSIZE : (g + 1) * GSIZE] = 1.0 / GSIZE
    return [
        {
            "x": xr[i * BPC : (i + 1) * BPC],
            "wt": wt,
            "gamma": gamma,
            "beta": beta,
            "agg": agg,
        }
        for i in range(N_CORES)
    ]


def kernel(x, weight, gamma, beta):
    x = np.asarray(x, dtype=np.float32)
    weight = np.asarray(weight, dtype=np.float32)
    assert x.shape == (B, CIN, H, W)
    nc = _get_program()
    in_maps = _make_in_maps(x, weight, gamma, beta)
    res = run_bass_kernel_spmd(nc, in_maps, core_ids=list(range(N_CORES)))
    out = np.concatenate([r["out"] for r in res.results], axis=0)
    return out.reshape(B, COUT, H, W)

